# revision 4
# baseline (speedup 1.0000x reference)
"""Multi-head attention (S=4096, D=1024, H=16, dk=64) on 8 TRN2 cores.

Sharding: tensor-parallel over heads — core c computes heads 2c, 2c+1 fully
(QKV projection, softmax attention, out-projection partial), host sums the
8 partial out-projections (the "all-reduce").

Device dataflow per core (all matmuls bf16, fp32 PSUM):
  xT [D, S] (host-pretransposed bf16)
  QT2/KT2 [128, S]   = (W.T @ xT) + bias    (rows 0-63 head0, 64-127 head1;
                                             Q and its bias pre-scaled 1/8)
  VT2 [128, S] -> PE-transpose -> V2 [S, 128]
  per q-chunk (512 q) x head:
    scoresT [k,q] psum groups of 4/3 k-chunks -> one ACT Exp per group
      -> E bf16 tiles (no max subtraction: |scores| < ~3, fp32-safe)
    den_acc += E (DVE), attn.V accumulates col-packed into one psum bank
  den: PE-transpose den_acc -> DVE reduce -> reciprocal -> DRAM round-trip
       broadcast to R [128(head-dims), 512(q)]
  CT [hd, q] = attnV_psum * R  (bf16)
  out[s, :]  = CT.T @ WO + b_out_broadcast  (b_out only on core 0; b_v folded
               into b_out on host)
"""

import numpy as np
import ml_dtypes
from contextlib import ExitStack

import concourse.bacc as bacc
import concourse.bass as bass
import concourse.tile as tile
from concourse import mybir
from concourse.bass_utils import run_bass_kernel_spmd
from concourse.masks import make_identity

BF16 = mybir.dt.bfloat16
F32 = mybir.dt.float32

S = 4096          # sequence length
D = 1024          # model dim
HPC = 2           # heads per core
DK = 64           # head dim
HD = HPC * DK     # 128 head-dims per core
NQ = 8            # q-chunks of 512
QW = 512          # q-chunk width
KC = 32           # k-chunks of 128
DC = 8            # d-chunks of 128
N_CORES = 8

# k-chunk groups per (q, h): sizes alternate 4 (ping: 4 psum banks) and
# 3 (pong: 3 banks); 4+3+4+3+4+3+4+3+4 = 32
GROUP_SIZES = [4, 3, 4, 3, 4, 3, 4, 3, 4]
GROUP_STARTS = [0, 4, 7, 11, 14, 18, 21, 25, 28]


def build_nc():
    nc = bacc.Bacc("TRN2", target_bir_lowering=False)

    xt = nc.declare_dram_parameter("xt", [D, S], BF16, isOutput=False)
    wq = nc.declare_dram_parameter("wq", [D, HD], BF16, isOutput=False)
    wk = nc.declare_dram_parameter("wk", [D, HD], BF16, isOutput=False)
    wv = nc.declare_dram_parameter("wv", [D, HD], BF16, isOutput=False)
    wo = nc.declare_dram_parameter("wo", [HD, D], BF16, isOutput=False)
    bq = nc.declare_dram_parameter("bq", [HD, 1], F32, isOutput=False)
    bk = nc.declare_dram_parameter("bk", [HD, 1], F32, isOutput=False)
    bob = nc.declare_dram_parameter("bob", [D], F32, isOutput=False)
    out = nc.declare_dram_parameter("out", [S, D], F32, isOutput=True)

    with tile.TileContext(nc) as tc, ExitStack() as ctx:
        consts = ctx.enter_context(tc.tile_pool(name="consts", bufs=1))
        persist = ctx.enter_context(tc.tile_pool(name="persist", bufs=1))
        dram = ctx.enter_context(tc.tile_pool(name="dram", bufs=1, space="DRAM"))

        ident_bf = consts.tile([128, 128], BF16)
        make_identity(nc, ident_bf)
        ident_f = consts.tile([128, 128], F32)
        make_identity(nc, ident_f)

        # weights / biases to SBUF
        wq_sb = consts.tile([128, DC, 128], BF16)
        wk_sb = consts.tile([128, DC, 128], BF16)
        wv_sb = consts.tile([128, DC, 128], BF16)
        for d in range(DC):
            nc.sync.dma_start(out=wq_sb[:, d, :], in_=wq[d * 128:(d + 1) * 128, :])
            nc.sync.dma_start(out=wk_sb[:, d, :], in_=wk[d * 128:(d + 1) * 128, :])
            nc.sync.dma_start(out=wv_sb[:, d, :], in_=wv[d * 128:(d + 1) * 128, :])
        wo_sb = consts.tile([128, D], BF16)
        nc.sync.dma_start(out=wo_sb, in_=wo[:, :])
        bq_sb = consts.tile([128, 1], F32)
        bk_sb = consts.tile([128, 1], F32)
        nc.sync.dma_start(out=bq_sb, in_=bq[:, :])
        nc.sync.dma_start(out=bk_sb, in_=bk[:, :])
        bob_sb = consts.tile([128, D], F32)
        nc.sync.dma_start(out=bob_sb,
                          in_=bass.AP(tensor=bob, offset=0, ap=[[0, 128], [1, D]]))

        QT2 = persist.tile([128, S], BF16)
        KT2 = persist.tile([128, S], BF16)
        V2 = persist.tile([128, KC, 128], BF16)
        CT2 = persist.tile([128, S], BF16)

        scratch = dram.tile([2 * NQ, QW], F32)

        # ---------------- Phase 1: projections ----------------
        with ExitStack() as ph1:
            xt_pool = ph1.enter_context(tc.tile_pool(name="xt", bufs=1))
            vt_pool = ph1.enter_context(tc.tile_pool(name="vt", bufs=1))

            xt_sb = xt_pool.tile([128, DC, S], BF16)
            for d in range(DC):
                nc.sync.dma_start(out=xt_sb[:, d, :], in_=xt[d * 128:(d + 1) * 128, :])

            VT2 = vt_pool.tile([128, S], BF16)

            # KT2 first (scores need all of K before q0), then QT2, then VT2
            with tc.tile_pool(name="mm1", bufs=2, space="PSUM") as mm1:
                for w_sb, bias_sb, dst in ((wk_sb, bk_sb, KT2), (wq_sb, bq_sb, QT2),
                                           (wv_sb, None, VT2)):
                    for sg in range(2):  # halves of S (2048 cols = 4 psum banks)
                        ps = mm1.tile([128, 2048], F32, tag="mm1")
                        for s4 in range(4):
                            for d in range(DC):
                                nc.tensor.matmul(
                                    ps[:, s4 * 512:(s4 + 1) * 512],
                                    w_sb[:, d, :],
                                    xt_sb[:, d, (sg * 4 + s4) * 512:(sg * 4 + s4 + 1) * 512],
                                    start=(d == 0), stop=(d == DC - 1))
                        half = dst[:, sg * 2048:(sg + 1) * 2048]
                        if bias_sb is None:
                            nc.vector.tensor_copy(out=half, in_=ps)
                        else:
                            nc.vector.tensor_scalar(out=half, in0=ps, scalar1=bias_sb,
                                                    scalar2=None, op0=mybir.AluOpType.add)

            # V2 = VT2 transposed per 128-chunk
            with tc.tile_pool(name="tr1", bufs=2, space="PSUM") as tr1:
                for c in range(KC):
                    tp = tr1.tile([128, 128], BF16, tag="tr1")
                    nc.tensor.transpose(tp, VT2[:, c * 128:(c + 1) * 128], ident_bf)
                    nc.vector.tensor_copy(out=V2[:, c, :], in_=tp)

        # ---------------- Phase 2: attention ----------------
        ping = ctx.enter_context(tc.tile_pool(name="ping", bufs=1, space="PSUM"))
        pong = ctx.enter_context(tc.tile_pool(name="pong", bufs=1, space="PSUM"))
        avp = ctx.enter_context(tc.tile_pool(name="avp", bufs=1, space="PSUM"))
        e_pool = ctx.enter_context(tc.tile_pool(name="epool", bufs=6))
        den_pool = ctx.enter_context(tc.tile_pool(name="den", bufs=4))
        small = ctx.enter_context(tc.tile_pool(name="small", bufs=4))
        r_pool = ctx.enter_context(tc.tile_pool(name="rpool", bufs=2))
        out_pool = ctx.enter_context(tc.tile_pool(name="outp", bufs=3))

        state = {}

        def emit_scores_exp_den(q, h, gi):
            gsz = GROUP_SIZES[gi]
            gst = GROUP_STARTS[gi]
            pool = ping if gsz == 4 else pong
            ps = pool.tile([128, gsz * 512], F32, tag=pool.name)
            hsl = slice(64 * h, 64 * (h + 1))
            for cl in range(gsz):
                c = gst + cl
                nc.tensor.matmul(
                    ps[:, cl * 512:(cl + 1) * 512],
                    KT2[hsl, c * 128:(c + 1) * 128],
                    QT2[hsl, q * QW:(q + 1) * QW],
                    start=True, stop=True)
            e_sb = e_pool.tile([128, gsz * 512], BF16, tag="e")
            nc.scalar.activation(out=e_sb, in_=ps,
                                 func=mybir.ActivationFunctionType.Exp)
            den_acc = state[("den", h)]
            for cl in range(gsz):
                sl = e_sb[:, cl * 512:(cl + 1) * 512]
                if gi == 0 and cl == 0:
                    nc.vector.tensor_copy(out=den_acc, in_=sl)
                else:
                    nc.vector.tensor_add(out=den_acc, in0=den_acc, in1=sl)
            state[("e", h, gi)] = e_sb

        def emit_attnv(q, h, gi):
            gsz = GROUP_SIZES[gi]
            gst = GROUP_STARTS[gi]
            e_sb = state.pop(("e", h, gi))
            av = state[("avp", q)]
            for cl in range(gsz):
                c = gst + cl
                nc.tensor.matmul(
                    av[64 * h:64 * (h + 1), :],
                    V2[:, c, 64 * h:64 * (h + 1)],
                    e_sb[:, cl * 512:(cl + 1) * 512],
                    start=(c == 0), stop=(c == KC - 1),
                    tile_position=(0, 64 * h))

        def emit_epilogue(q):
            av = state.pop(("avp", q))
            # den -> reciprocal -> scratch row (transposed store)
            for h in range(2):
                den_acc = state.pop(("den", h))
                denT = pong.tile([128, 512], F32, tag=pong.name)
                denq = small.tile([128, 4], F32, tag="denq")
                for j in range(4):
                    nc.tensor.transpose(denT[:, j * 128:(j + 1) * 128],
                                        den_acc[:, j * 128:(j + 1) * 128], ident_f)
                    nc.vector.reduce_sum(denq[:, j:j + 1],
                                         denT[:, j * 128:(j + 1) * 128],
                                         axis=mybir.AxisListType.X)
                recq = small.tile([128, 4], F32, tag="recq")
                nc.vector.reciprocal(out=recq, in_=denq)
                row = q * 2 + h
                nc.gpsimd.dma_start(
                    out=bass.AP(tensor=scratch.tensor,
                                offset=scratch.offset + row * QW,
                                ap=[[1, 128], [128, 4]]),
                    in_=recq)
            # broadcast reciprocals: R[p, q'] = recip_{head(p)}[q']
            R = r_pool.tile([128, QW], F32, tag="r")
            for h in range(2):
                nc.gpsimd.dma_start(
                    out=R[64 * h:64 * (h + 1), :],
                    in_=bass.AP(tensor=scratch.tensor,
                                offset=scratch.offset + (q * 2 + h) * QW,
                                ap=[[0, 64], [1, QW]]))
            nc.vector.tensor_tensor(out=CT2[:, q * QW:(q + 1) * QW],
                                    in0=av, in1=R, op=mybir.AluOpType.mult)
            # out-projection for this q-chunk (4 s-chunks of 128)
            for sl4 in range(4):
                s = q * 4 + sl4
                o_sb = out_pool.tile([128, D], F32, tag="o")
                for nh in range(2):
                    op_ps = avp.tile([128, 512], F32, tag="avp")
                    nc.tensor.matmul(op_ps, CT2[:, s * 128:(s + 1) * 128],
                                     wo_sb[:, nh * 512:(nh + 1) * 512],
                                     start=True, stop=True)
                    nc.vector.tensor_tensor(
                        out=o_sb[:, nh * 512:(nh + 1) * 512], in0=op_ps,
                        in1=bob_sb[:, nh * 512:(nh + 1) * 512],
                        op=mybir.AluOpType.add)
                nc.sync.dma_start(out=out[s * 128:(s + 1) * 128, :], in_=o_sb)

        stream = [(q, h, gi) for q in range(NQ) for h in range(2)
                  for gi in range(len(GROUP_SIZES))]
        pending = None
        for (q, h, gi) in stream:
            if h == 0 and gi == 0:
                state[("avp", q)] = avp.tile([128, QW], F32, tag="avp", name=f"avt{q}")
            if gi == 0:
                state[("den", h)] = den_pool.tile([128, QW], F32, tag="den", name=f"den{h}")
            emit_scores_exp_den(q, h, gi)
            if pending is not None:
                emit_attnv(*pending)
            pending = (q, h, gi)
            if h == 1 and gi == len(GROUP_SIZES) - 1:
                emit_attnv(*pending)
                pending = None
                emit_epilogue(q)

    nc.finalize()
    return nc


_CACHE = {}


def _get_nc():
    if "nc" not in _CACHE:
        _CACHE["nc"] = build_nc()
    return _CACHE["nc"]


def _prep_in_maps(x, w_qkv, b_qkv, w_out, b_out):
    bf = ml_dtypes.bfloat16
    xtv = np.ascontiguousarray(x.reshape(S, D).T).astype(bf)
    bv = b_qkv[2 * D:3 * D].astype(np.float64)
    bob0 = (b_out.astype(np.float64) + bv @ w_out.astype(np.float64)).astype(np.float32)
    in_maps = []
    for c in range(N_CORES):
        hs = slice(c * HD, (c + 1) * HD)
        in_maps.append({
            "xt": xtv,
            "wq": np.ascontiguousarray(w_qkv[:, hs].astype(np.float64) / 8.0).astype(bf),
            "wk": np.ascontiguousarray(w_qkv[:, D:2 * D][:, hs]).astype(bf),
            "wv": np.ascontiguousarray(w_qkv[:, 2 * D:3 * D][:, hs]).astype(bf),
            "wo": np.ascontiguousarray(w_out[hs, :]).astype(bf),
            "bq": np.ascontiguousarray(b_qkv[hs].astype(np.float64) / 8.0
                                       ).astype(np.float32).reshape(HD, 1),
            "bk": np.ascontiguousarray(b_qkv[D:2 * D][hs]).astype(np.float32
                                                                  ).reshape(HD, 1),
            "bob": bob0 if c == 0 else np.zeros(D, np.float32),
        })
    return in_maps


def kernel(x, w_qkv, b_qkv, w_out, b_out):
    x = np.asarray(x, np.float32)
    w_qkv = np.asarray(w_qkv, np.float32)
    b_qkv = np.asarray(b_qkv, np.float32)
    w_out = np.asarray(w_out, np.float32)
    b_out = np.asarray(b_out, np.float32)

    nc = _get_nc()
    in_maps = _prep_in_maps(x, w_qkv, b_qkv, w_out, b_out)
    res = run_bass_kernel_spmd(nc, in_maps, list(range(N_CORES)))
    acc = np.zeros((S, D), np.float64)
    for r in res.results:
        acc += r["out"].astype(np.float64)
    return acc.astype(np.float32).reshape(1, S, D)


# revision 10
# speedup vs baseline: 1.3059x; 1.3059x over previous
"""Multi-head attention (S=4096, D=1024, H=16, dk=64) on 8 TRN2 cores.

Sharding: tensor-parallel over heads — core c computes heads 2c, 2c+1 fully
(QKV projection, softmax attention, out-projection partial), host sums the
8 partial out-projections (the "all-reduce").

Device dataflow per core (all matmuls bf16, fp32 PSUM):
  xT [D, S] (host-pretransposed bf16)
  QT2/KT2 [128, S]   = (W.T @ xT) + bias    (rows 0-63 head0, 64-127 head1;
                                             Q and its bias pre-scaled 1/8)
  VT2 [128, S] -> PE-transpose -> V2 [S, 128]
  per q-chunk (512 q) x head:
    scoresT [k,q] psum groups of 4/3 k-chunks -> one ACT Exp per group
      -> E bf16 tiles (no max subtraction: |scores| < ~3, fp32-safe)
    den_acc += E (DVE), attn.V accumulates col-packed into one psum bank
  den: PE-transpose den_acc -> DVE reduce -> reciprocal -> DRAM round-trip
       broadcast to R [128(head-dims), 512(q)]
  CT [hd, q] = attnV_psum * R  (bf16)
  out[s, :]  = CT.T @ WO + b_out_broadcast  (b_out only on core 0; b_v folded
               into b_out on host)
"""

import numpy as np
import ml_dtypes
from contextlib import ExitStack

import concourse.bacc as bacc
import concourse.bass as bass
import concourse.tile as tile
from concourse import mybir
from concourse.bass_utils import run_bass_kernel_spmd
from concourse.masks import make_identity

BF16 = mybir.dt.bfloat16
F32 = mybir.dt.float32

S = 4096          # sequence length
D = 1024          # model dim
HPC = 2           # heads per core
DK = 64           # head dim
HD = HPC * DK     # 128 head-dims per core
NQ = 8            # q-chunks of 512
QW = 512          # q-chunk width
KC = 32           # k-chunks of 128
DC = 8            # d-chunks of 128
N_CORES = 8

# k-chunk groups per (q, h): ping/pong pools of 3 psum banks each; the
# attn.V psums (one [65, 512] bank per head, bufs=2) take the other 2 banks.
GROUP_SIZES = [3, 3, 3, 3, 3, 3, 3, 3, 3, 3, 2]
GROUP_STARTS = [0, 3, 6, 9, 12, 15, 18, 21, 24, 27, 30]


def build_nc():
    nc = bacc.Bacc("TRN2", target_bir_lowering=False)

    xt = nc.declare_dram_parameter("xt", [D, S], BF16, isOutput=False)
    wq = nc.declare_dram_parameter("wq", [D, HD], BF16, isOutput=False)
    wk = nc.declare_dram_parameter("wk", [D, HD], BF16, isOutput=False)
    wv = nc.declare_dram_parameter("wv", [D, HD], BF16, isOutput=False)
    wo = nc.declare_dram_parameter("wo", [HD, D], BF16, isOutput=False)
    bq = nc.declare_dram_parameter("bq", [HD, 1], F32, isOutput=False)
    bk = nc.declare_dram_parameter("bk", [HD, 1], F32, isOutput=False)
    bob = nc.declare_dram_parameter("bob", [D], F32, isOutput=False)
    out = nc.declare_dram_parameter("out", [S, D], F32, isOutput=True)

    with tile.TileContext(nc) as tc, ExitStack() as ctx:
        consts = ctx.enter_context(tc.tile_pool(name="consts", bufs=1))
        persist = ctx.enter_context(tc.tile_pool(name="persist", bufs=1))
        dram = ctx.enter_context(tc.tile_pool(name="dram", bufs=1, space="DRAM"))

        ident_bf = consts.tile([128, 128], BF16)
        make_identity(nc, ident_bf)
        ident_f = consts.tile([128, 128], F32)
        make_identity(nc, ident_f)

        # weights / biases to SBUF
        wq_sb = consts.tile([128, DC, 128], BF16)
        wk_sb = consts.tile([128, DC, 128], BF16)
        wv_sb = consts.tile([128, DC, 128], BF16)
        for d in range(DC):
            nc.sync.dma_start(out=wq_sb[:, d, :], in_=wq[d * 128:(d + 1) * 128, :])
            nc.sync.dma_start(out=wk_sb[:, d, :], in_=wk[d * 128:(d + 1) * 128, :])
            nc.sync.dma_start(out=wv_sb[:, d, :], in_=wv[d * 128:(d + 1) * 128, :])
        wo_h0 = consts.tile([64, D], BF16)
        wo_h1 = consts.tile([64, D], BF16)
        nc.sync.dma_start(out=wo_h0, in_=wo[0:64, :])
        nc.sync.dma_start(out=wo_h1, in_=wo[64:128, :])
        bq_sb = consts.tile([128, 1], F32)
        bk_sb = consts.tile([128, 1], F32)
        nc.sync.dma_start(out=bq_sb, in_=bq[:, :])
        nc.sync.dma_start(out=bk_sb, in_=bk[:, :])
        bob_sb = consts.tile([128, D], F32)
        nc.sync.dma_start(out=bob_sb,
                          in_=bass.AP(tensor=bob, offset=0, ap=[[0, 128], [1, D]]))

        QT2 = persist.tile([128, S], BF16)
        KT2 = persist.tile([128, S], BF16)
        # V2[:, c, 65h : 65h+64] = V rows of k-chunk c for head h, col 65h+64
        # is ones (softmax denominator via an extra matmul output row)
        V2 = persist.tile([128, KC, 130], BF16)
        CT_h0 = persist.tile([64, S], BF16)
        CT_h1 = persist.tile([64, S], BF16)

        scratch = dram.tile([2 * NQ, QW], F32)

        # ---------------- Phase 1: projections ----------------
        with ExitStack() as ph1:
            xt_pool = ph1.enter_context(tc.tile_pool(name="xt", bufs=1))
            vt_pool = ph1.enter_context(tc.tile_pool(name="vt", bufs=1))

            xt_sb = xt_pool.tile([128, DC, S], BF16)
            for sg in range(2):  # s-halves first so KT2/QT2 half 0 start early
                for d in range(DC):
                    nc.sync.dma_start(
                        out=xt_sb[:, d, sg * 2048:(sg + 1) * 2048],
                        in_=xt[d * 128:(d + 1) * 128, sg * 2048:(sg + 1) * 2048])

            VT2 = vt_pool.tile([128, S], BF16)

            # emission order: K/Q of s-half 0 first so scores for q0 can start
            # while the rest of the projections still run
            with tc.tile_pool(name="mm1", bufs=2, space="PSUM") as mm1:
                jobs = [(wk_sb, bk_sb, KT2, 0), (wq_sb, bq_sb, QT2, 0),
                        (wk_sb, bk_sb, KT2, 1), (wq_sb, bq_sb, QT2, 1),
                        (wv_sb, None, VT2, 0), (wv_sb, None, VT2, 1)]
                for w_sb, bias_sb, dst, sg in jobs:
                    ps = mm1.tile([128, 2048], F32, tag="mm1")
                    for s4 in range(4):
                        for d in range(DC):
                            nc.tensor.matmul(
                                ps[:, s4 * 512:(s4 + 1) * 512],
                                w_sb[:, d, :],
                                xt_sb[:, d, (sg * 4 + s4) * 512:(sg * 4 + s4 + 1) * 512],
                                start=(d == 0), stop=(d == DC - 1))
                    half = dst[:, sg * 2048:(sg + 1) * 2048]
                    if bias_sb is None:
                        nc.vector.tensor_copy(out=half, in_=ps)
                    else:
                        nc.vector.tensor_scalar(out=half, in0=ps, scalar1=bias_sb,
                                                scalar2=None, op0=mybir.AluOpType.add)

            # V2 = VT2 transposed per 128-chunk, heads split around ones cols
            nc.vector.memset(
                V2.rearrange("p c (g o) -> p (c g) o", o=65)[:, :, 64:65], 1.0)
            with tc.tile_pool(name="tr1", bufs=2, space="PSUM") as tr1:
                for c in range(KC):
                    tp = tr1.tile([128, 128], BF16, tag="tr1")
                    nc.tensor.transpose(tp, VT2[:, c * 128:(c + 1) * 128], ident_bf)
                    nc.vector.tensor_copy(
                        out=V2[:, c, :].rearrange("p (g o) -> p g o", o=65)[:, :, 0:64],
                        in_=tp.rearrange("p (g o) -> p g o", o=64))

        # ---------------- Phase 2: attention ----------------
        ping = ctx.enter_context(tc.tile_pool(name="ping", bufs=1, space="PSUM"))
        pong = ctx.enter_context(tc.tile_pool(name="pong", bufs=1, space="PSUM"))
        avp = ctx.enter_context(tc.tile_pool(name="avp", bufs=2, space="PSUM"))
        e_pool = ctx.enter_context(tc.tile_pool(name="epool", bufs=6))
        small = ctx.enter_context(tc.tile_pool(name="small", bufs=4))
        r_pool = ctx.enter_context(tc.tile_pool(name="rpool", bufs=4))
        out_pool = ctx.enter_context(tc.tile_pool(name="outp", bufs=3))

        state = {}
        NG = len(GROUP_SIZES)

        def emit_scores_exp(q, h, gi):
            gsz = GROUP_SIZES[gi]
            gst = GROUP_STARTS[gi]
            pool = ping if gi % 2 == 0 else pong
            ps = pool.tile([128, gsz * 512], F32, tag=pool.name, name=f"sc{q}_{h}_{gi}")
            hsl = slice(64 * h, 64 * (h + 1))
            for cl in range(gsz):
                c = gst + cl
                nc.tensor.matmul(
                    ps[:, cl * 512:(cl + 1) * 512],
                    KT2[hsl, c * 128:(c + 1) * 128],
                    QT2[hsl, q * QW:(q + 1) * QW],
                    start=True, stop=True)
            e_sb = e_pool.tile([128, gsz * 512], BF16, tag="e", name=f"e{q}_{h}_{gi}")
            nc.scalar.activation(out=e_sb, in_=ps,
                                 func=mybir.ActivationFunctionType.Exp)
            state[("e", h, gi)] = e_sb

        def emit_attnv(q, h, gi):
            gsz = GROUP_SIZES[gi]
            gst = GROUP_STARTS[gi]
            e_sb = state.pop(("e", h, gi))
            av = state[("avp", q, h)]
            for cl in range(gsz):
                c = gst + cl
                nc.tensor.matmul(
                    av,
                    V2[:, c, 65 * h:65 * h + 65],
                    e_sb[:, cl * 512:(cl + 1) * 512],
                    start=(c == 0), stop=(c == KC - 1))

        def emit_epilogue(q, h):
            av = state.pop(("avp", q, h))
            row = q * 2 + h
            rec = small.tile([1, QW], F32, tag="rec", name=f"rec{q}_{h}")
            nc.vector.reciprocal(out=rec, in_=av[64:65, :])
            nc.gpsimd.dma_start(
                out=bass.AP(tensor=scratch.tensor,
                            offset=scratch.offset + row * QW,
                            ap=[[0, 1], [1, QW]]),
                in_=rec)
            R = r_pool.tile([64, QW], F32, tag="r", name=f"r{q}_{h}")
            nc.gpsimd.dma_start(
                out=R,
                in_=bass.AP(tensor=scratch.tensor,
                            offset=scratch.offset + row * QW,
                            ap=[[0, 64], [1, QW]]))
            ct = CT_h0 if h == 0 else CT_h1
            nc.vector.tensor_tensor(out=ct[:, q * QW:(q + 1) * QW],
                                    in0=av[0:64, :], in1=R,
                                    op=mybir.AluOpType.mult)
            if h == 1:
                emit_outproj(q)

        def emit_outproj(q):
            for sl4 in range(4):
                s = q * 4 + sl4
                o_sb = out_pool.tile([128, D], F32, tag="o", name=f"o{s}")
                for nh in range(2):
                    op_ps = avp.tile([128, 512], F32, tag="avp", name=f"op{s}_{nh}")
                    nc.tensor.matmul(op_ps, CT_h0[:, s * 128:(s + 1) * 128],
                                     wo_h0[:, nh * 512:(nh + 1) * 512],
                                     start=True, stop=False)
                    nc.tensor.matmul(op_ps, CT_h1[:, s * 128:(s + 1) * 128],
                                     wo_h1[:, nh * 512:(nh + 1) * 512],
                                     start=False, stop=True)
                    nc.vector.tensor_tensor(
                        out=o_sb[:, nh * 512:(nh + 1) * 512], in0=op_ps,
                        in1=bob_sb[:, nh * 512:(nh + 1) * 512],
                        op=mybir.AluOpType.add)
                nc.sync.dma_start(out=out[s * 128:(s + 1) * 128, :], in_=o_sb)

        stream = [(q, h, gi) for q in range(NQ) for h in range(2)
                  for gi in range(NG)]
        pending = None
        for (q, h, gi) in stream:
            if gi == 0:
                state[("avp", q, h)] = avp.tile([65, QW], F32, tag="avp",
                                                name=f"avt{q}_{h}")
            emit_scores_exp(q, h, gi)
            if pending is not None:
                emit_attnv(*pending)
                if pending[2] == NG - 1:
                    emit_epilogue(pending[0], pending[1])
            pending = (q, h, gi)
        emit_attnv(*pending)
        emit_epilogue(pending[0], pending[1])

    nc.finalize()
    return nc


_CACHE = {}


def _get_nc():
    if "nc" not in _CACHE:
        _CACHE["nc"] = build_nc()
    return _CACHE["nc"]


def _prep_in_maps(x, w_qkv, b_qkv, w_out, b_out):
    bf = ml_dtypes.bfloat16
    xtv = np.ascontiguousarray(x.reshape(S, D).T).astype(bf)
    bv = b_qkv[2 * D:3 * D].astype(np.float64)
    bob0 = (b_out.astype(np.float64) + bv @ w_out.astype(np.float64)).astype(np.float32)
    in_maps = []
    for c in range(N_CORES):
        hs = slice(c * HD, (c + 1) * HD)
        in_maps.append({
            "xt": xtv,
            "wq": np.ascontiguousarray(w_qkv[:, hs].astype(np.float64) / 8.0).astype(bf),
            "wk": np.ascontiguousarray(w_qkv[:, D:2 * D][:, hs]).astype(bf),
            "wv": np.ascontiguousarray(w_qkv[:, 2 * D:3 * D][:, hs]).astype(bf),
            "wo": np.ascontiguousarray(w_out[hs, :]).astype(bf),
            "bq": np.ascontiguousarray(b_qkv[hs].astype(np.float64) / 8.0
                                       ).astype(np.float32).reshape(HD, 1),
            "bk": np.ascontiguousarray(b_qkv[D:2 * D][hs]).astype(np.float32
                                                                  ).reshape(HD, 1),
            "bob": bob0 if c == 0 else np.zeros(D, np.float32),
        })
    return in_maps


def kernel(x, w_qkv, b_qkv, w_out, b_out):
    x = np.asarray(x, np.float32)
    w_qkv = np.asarray(w_qkv, np.float32)
    b_qkv = np.asarray(b_qkv, np.float32)
    w_out = np.asarray(w_out, np.float32)
    b_out = np.asarray(b_out, np.float32)

    nc = _get_nc()
    in_maps = _prep_in_maps(x, w_qkv, b_qkv, w_out, b_out)
    res = run_bass_kernel_spmd(nc, in_maps, list(range(N_CORES)))
    acc = np.zeros((S, D), np.float64)
    for r in res.results:
        acc += r["out"].astype(np.float64)
    return acc.astype(np.float32).reshape(1, S, D)
